# revision 30
# baseline (speedup 1.0000x reference)
"""Multi-head attention (S=2048, B=4, H=1024, NH=16) on 8 Trainium2 NeuronCores.

Sharding: each core handles 2 batches x 4 heads (batch pairs balanced by
valid length; tensor-parallel over heads). bf16 matmuls, fp32 accumulate.

No mask bias anywhere: the raw q/k/v inputs are zeroed at padded positions,
so projected k and v are exactly zero there, scores for padded keys are
exactly 0 and exp(0)=1 -- each padded key contributes exactly 1.0 to Z
(subtracted via the per-slot npad input) and nothing to P@V.

Per (qc, p, kc) the PE work is three ~one-stream slots:
  - scores: 4 quadrant-tiled MMs (2 heads x 2 k-halves) sharing two
    parallel column-group streams
  - PV col-pair (h0 || h1), Z col-pair (ones-matmul, h0 || h1)
exp runs on ScalarE (scale only). Scores live in two separate per-slot
PSUM tiles -- Tile's view-overlap dep tracking is per-tile, so score
writes of one slot never false-WAR against exp reads of the other.

Schedule: slot-0 k/v projections run ic-outer in a full-PSUM phase that
overlaps the initial input DMA; slot-0 attention (sequential p, 2 score
slots) starts as soon as q-chunk 0 is projected, with the remaining
slot-0 q chunks and all slot-1 DMAs/projections fed into its emission
stream; slot-1 attention interleaves both head-pair chains so ScalarE
stays packed (chain p owns score tile p). Each qc's Wo (col-paired MMs,
PSUM carved from the score tiles) is deferred into the next qc's kc loop.
Input DMAs are spread across the sync/scalar/gpsimd queues; slot-1 output
DMAs go on the then-idle sync queue. The last q-chunk of each slot is
pruned to the valid width (rounded to 64). Normalization:
rz = recip(Z - npad); ab = attn * rz; padded-query columns are zeroed
post-Wo (ysb = yps * kqr). Host sums 4 head-quad partials per batch.
"""
import sys

if "/opt/trn_rl_repo" not in sys.path:
    sys.path.insert(0, "/opt/trn_rl_repo")

import math
from itertools import permutations

import ml_dtypes
import numpy as np

import concourse.bass as bass
import concourse.mybir as mybir
import concourse.tile as tile
from concourse import bacc
from concourse.bass_utils import run_bass_kernel_spmd

S, B, H, NH, DK = 2048, 4, 1024, 16, 64
N_CORES = 8
BF16 = mybir.dt.bfloat16
F32 = mybir.dt.float32
NPBF16 = ml_dtypes.bfloat16
SCALE = 1.0 / math.sqrt(DK)

_prog_cache: dict = {}


def _build_program(NQ, NK, W):
    """One SPMD program. Per slot s: NQ[s] 512-wide q chunks (last pruned to
    W[s]), NK[s] 128-wide k chunks. Slot 0 should be the smaller workload."""
    NSCK = [(nk * 128 + 511) // 512 for nk in NK]
    KW = [nk * 128 - (nsc - 1) * 512 for nk, nsc in zip(NK, NSCK)]  # last k-sc width
    QW = [(nq - 1) * 512 + w for nq, w in zip(NQ, W)]  # loaded q extent
    nc = bacc.Bacc("TRN2", target_bir_lowering=False, debug=False,
                   num_devices=N_CORES)

    d_in = {}
    for s in range(2):
        d_in[f"qT{s}"] = nc.dram_tensor(f"qT{s}", [H, S], BF16, kind="ExternalInput")
        d_in[f"kT{s}"] = nc.dram_tensor(f"kT{s}", [H, S], BF16, kind="ExternalInput")
        d_in[f"vT{s}"] = nc.dram_tensor(f"vT{s}", [H, S], BF16, kind="ExternalInput")
        d_in[f"kq{s}"] = nc.dram_tensor(f"kq{s}", [4, 512], F32, kind="ExternalInput")
        d_in[f"pd{s}"] = nc.dram_tensor(f"pd{s}", [1, 1], F32, kind="ExternalInput")
    d_in["wqT"] = nc.dram_tensor("wqT", [H, 256], BF16, kind="ExternalInput")
    d_in["wkT"] = nc.dram_tensor("wkT", [H, 256], BF16, kind="ExternalInput")
    d_in["wvT"] = nc.dram_tensor("wvT", [H, 256], BF16, kind="ExternalInput")
    d_in["woT"] = nc.dram_tensor("woT", [256, H], BF16, kind="ExternalInput")
    d_out = [nc.dram_tensor(f"y{s}", [H, S], BF16, kind="ExternalOutput")
             for s in range(2)]

    with tile.TileContext(nc) as tc:
        with tc.tile_pool(name="wpool", bufs=1) as wpool, \
             tc.tile_pool(name="in8", bufs=1) as in8, \
             tc.tile_pool(name="persist", bufs=1) as persist, \
             tc.tile_pool(name="probs", bufs=4) as probsp, \
             tc.tile_pool(name="small", bufs=2) as small, \
             tc.tile_pool(name="att", bufs=3) as attp, \
             tc.tile_pool(name="yst", bufs=6) as ystp:

            # --- constants / weights ---
            wq = [wpool.tile([128, 256], BF16, name=f"wq{i}", tag=f"wq{i}")
                  for i in range(8)]
            wk = [wpool.tile([128, 256], BF16, name=f"wk{i}", tag=f"wk{i}")
                  for i in range(8)]
            wv = [wpool.tile([128, 256], BF16, name=f"wv{i}", tag=f"wv{i}")
                  for i in range(8)]
            wo = [wpool.tile([128, H], BF16, name=f"wo{j}", tag=f"wo{j}")
                  for j in range(2)]
            for i in range(8):
                nc.gpsimd.dma_start(out=wk[i][:], in_=d_in["wkT"].ap()[i * 128:(i + 1) * 128, :])
                nc.gpsimd.dma_start(out=wv[i][:], in_=d_in["wvT"].ap()[i * 128:(i + 1) * 128, :])
                nc.gpsimd.dma_start(out=wq[i][:], in_=d_in["wqT"].ap()[i * 128:(i + 1) * 128, :])
            for j in range(2):
                nc.gpsimd.dma_start(out=wo[j][:], in_=d_in["woT"].ap()[j * 128:(j + 1) * 128, :])
            ones = wpool.tile([128, 64], BF16, name="ones", tag="ones")
            nc.vector.memset(ones[:], 1.0)
            npadt = [wpool.tile([128, 1], F32, name=f"npad{s}", tag=f"npad{s}")
                     for s in range(2)]
            for s in range(2):
                nc.gpsimd.dma_start(
                    out=npadt[s][:],
                    in_=bass.AP(tensor=d_in[f"pd{s}"], offset=0,
                                ap=[[0, 128], [1, 1]]))

            # --- persistent projection outputs ---
            qTp = [[persist.tile([128, NQ[s] * 512], BF16, name=f"qTp{s}_{p}",
                                 tag=f"qTp{s}_{p}")
                    for p in range(2)] for s in range(2)]
            kTp = [[persist.tile([128, NSCK[s] * 512], BF16, name=f"kTp{s}_{p}",
                                 tag=f"kTp{s}_{p}")
                    for p in range(2)] for s in range(2)]
            vp = [[persist.tile([128, 256], BF16, name=f"vp{s}_{st}", tag=f"vp{s}_{st}")
                   for st in range(NK[s])] for s in range(2)]

            def q_sc_width(s, sc):
                return 512 if sc < NQ[s] - 1 else W[s]

            def k_sc_width(s, sc):
                return 512 if sc < NSCK[s] - 1 else KW[s]

            def proj_units(s, pool, kinds=("k", "v", "q")):
                """Generator emitting slot-s input DMAs + projection groups
                in dependency-friendly order: k-ft0, v, q-sc0 (both ft),
                then k-ft1 and remaining q chunks. Yields a label after each
                unit; the caller may eager-drive to a label."""
                tiles = {}
                specs = {"k": (f"kT{s}", NK[s] * 128, nc.sync),
                         "v": (f"vT{s}", NK[s] * 128, nc.scalar if s == 0 else nc.gpsimd),
                         "q": (f"qT{s}", QW[s], nc.sync)}
                for kind in kinds:
                    dname, wdt, eng = specs[kind]
                    for ic in range(8):
                        it = in8.tile([128, wdt], BF16, name=f"pf{kind}{s}_{ic}",
                                      tag=f"pf{kind}{ic}")
                        eng.dma_start(
                            out=it[:],
                            in_=d_in[dname].ap()[ic * 128:(ic + 1) * 128, 0:wdt])
                        tiles[(kind, ic)] = it
                        yield "dma"
                gidx = [0]

                def qk_group(kind, wts, ft, sc, cw, outtiles):
                    pj = pool.tile([128, 512], F32,
                                   name=f"pj{s}{kind}_{ft}_{sc}",
                                   tag=f"pj{gidx[0] % 2}")
                    gidx[0] += 1
                    for ic in range(0, 8, 2):
                        for i2 in (ic, ic + 1):
                            nc.tensor.matmul(
                                out=pj[:, 0:cw],
                                lhsT=wts[i2][:, ft * 128:(ft + 1) * 128],
                                rhs=tiles[(kind, i2)][:, sc * 512:sc * 512 + cw],
                                start=(i2 == 0), stop=(i2 == 7))
                        yield "mm"
                    nc.vector.tensor_copy(
                        outtiles[ft][:, sc * 512:sc * 512 + cw], pj[:, 0:cw])
                    yield f"{kind}:{ft}:{sc}"

                def v_group(st):
                    pj = pool.tile([128, 512], F32, name=f"pjv{s}_{st}",
                                   tag=f"pj{gidx[0] % 2}")
                    gidx[0] += 1
                    for ic in range(0, 8, 2):
                        for i2 in (ic, ic + 1):
                            nc.tensor.matmul(
                                out=pj[:, 0:256],
                                lhsT=tiles[("v", i2)][:, st * 128:(st + 1) * 128],
                                rhs=wv[i2][:, :],
                                start=(i2 == 0), stop=(i2 == 7))
                        yield "mm"
                    if st % 2:
                        nc.scalar.copy(vp[s][st][:], pj[:, 0:256])
                    else:
                        nc.vector.tensor_copy(vp[s][st][:], pj[:, 0:256])
                    yield f"v:{st}"

                if "k" in kinds:
                    for sc in range(NSCK[s]):
                        yield from qk_group("k", wk, 0, sc, k_sc_width(s, sc), kTp[s])
                if "v" in kinds:
                    for st in range(NK[s]):
                        yield from v_group(st)
                yield from qk_group("q", wq, 0, 0, q_sc_width(s, 0), qTp[s])
                yield from qk_group("q", wq, 1, 0, q_sc_width(s, 0), qTp[s])
                if "k" in kinds:
                    for sc in range(NSCK[s]):
                        yield from qk_group("k", wk, 1, sc, k_sc_width(s, sc), kTp[s])
                for sc in range(1, NQ[s]):
                    yield from qk_group("q", wq, 0, sc, q_sc_width(s, sc), qTp[s])
                    yield from qk_group("q", wq, 1, sc, q_sc_width(s, sc), qTp[s])

            def emit_proj0_eager(pool):
                """Slot-0 k (both ft) and v projections, ic-outer so MMs
                start after the first input chunk lands; uses a full-PSUM
                scope. Inputs go to the shared in8 tiles."""
                s = 0
                kext = NK[s] * 128
                kt, vt = [], []
                for ic in range(8):
                    it = in8.tile([128, kext], BF16, name=f"pfk{s}_{ic}",
                                  tag=f"pfk{ic}")
                    nc.sync.dma_start(
                        out=it[:],
                        in_=d_in[f"kT{s}"].ap()[ic * 128:(ic + 1) * 128, 0:kext])
                    kt.append(it)
                for ic in range(8):
                    it = in8.tile([128, kext], BF16, name=f"pfv{s}_{ic}",
                                  tag=f"pfv{ic}")
                    nc.scalar.dma_start(
                        out=it[:],
                        in_=d_in[f"vT{s}"].ap()[ic * 128:(ic + 1) * 128, 0:kext])
                    vt.append(it)
                ps = {(ft, sc): pool.tile([128, 512], F32, name=f"e0k_{ft}_{sc}",
                                          tag=f"pj_{ft}_{sc}")
                      for ft in range(2) for sc in range(NSCK[s])}
                for ic in range(8):
                    for ft in range(2):
                        for sc in range(NSCK[s]):
                            cw = k_sc_width(s, sc)
                            nc.tensor.matmul(
                                out=ps[(ft, sc)][:, 0:cw],
                                lhsT=wk[ic][:, ft * 128:(ft + 1) * 128],
                                rhs=kt[ic][:, sc * 512:sc * 512 + cw],
                                start=(ic == 0), stop=(ic == 7))
                for ft in range(2):
                    for sc in range(NSCK[s]):
                        cw = k_sc_width(s, sc)
                        if ft == 0:
                            nc.vector.tensor_copy(
                                kTp[s][ft][:, sc * 512:sc * 512 + cw],
                                ps[(ft, sc)][:, 0:cw])
                        else:
                            nc.scalar.copy(
                                kTp[s][ft][:, sc * 512:sc * 512 + cw],
                                ps[(ft, sc)][:, 0:cw])
                for st0 in range(0, NK[s], 8):
                    sts = range(st0, min(st0 + 8, NK[s]))
                    psv = {st: pool.tile([128, 256], F32, name=f"e0v_{st}",
                                         tag=f"pj_{(st - st0) // 4}_{(st - st0) % 4}")
                           for st in sts}
                    for ic in range(8):
                        for st in sts:
                            nc.tensor.matmul(
                                out=psv[st][:],
                                lhsT=vt[ic][:, st * 128:(st + 1) * 128],
                                rhs=wv[ic][:, :],
                                start=(ic == 0), stop=(ic == 7))
                    for st in sts:
                        if st % 2:
                            nc.scalar.copy(vp[s][st][:], psv[st][:])
                        else:
                            nc.vector.tensor_copy(vp[s][st][:], psv[st][:])

            def mk_scores(s, sc_tiles, qc, w, hstride):
                def emit_scores(p, kc, sl, foff):
                    """4 quadrant MMs for (p, kc) into score tile sl at
                    offset foff, heads strided by hstride."""
                    for kh in range(2):
                        for h in range(2):
                            o = foff + h * hstride
                            nc.tensor.matmul(
                                out=sc_tiles[sl][kh * 64:(kh + 1) * 64, o:o + w],
                                lhsT=kTp[s][p][h * 64:(h + 1) * 64,
                                               kc * 128 + kh * 64:kc * 128 + (kh + 1) * 64],
                                rhs=qTp[s][p][h * 64:(h + 1) * 64,
                                              qc * 512:qc * 512 + w],
                                start=True, stop=True,
                                skip_group_check=True)
                return emit_scores

            def mk_pvz(s, w, attn, zps, hstride):
                def emit_pvz(p, kc, pr, poff):
                    first, last = kc == 0, kc == NK[s] - 1
                    for h in range(2):
                        nc.tensor.matmul(
                            out=attn[p][h * 64:(h + 1) * 64, 0:w],
                            lhsT=vp[s][kc][:, p * 128 + h * 64:p * 128 + (h + 1) * 64],
                            rhs=pr[:, poff + h * hstride:poff + h * hstride + w],
                            start=first, stop=last,
                            skip_group_check=True)
                    for h in range(2):
                        nc.tensor.matmul(
                            out=zps[p][h * 64:(h + 1) * 64, 0:w],
                            lhsT=ones[:, :],
                            rhs=pr[:, poff + h * hstride:poff + h * hstride + w],
                            start=first, stop=last,
                            skip_group_check=True)
                return emit_pvz

            def emit_norm(s, qc, p, w, attn, zps, att_sb):
                zadj = small.tile([128, 512], F32, name=f"za{s}_{qc}_{p}",
                                  tag="za")
                nc.vector.tensor_scalar_sub(
                    zadj[:, 0:w], zps[p][:, 0:w], npadt[s][:, 0:1])
                rz = small.tile([128, 512], F32, name=f"rz{s}_{qc}_{p}",
                                tag="rz")
                nc.vector.reciprocal_approx_fast(out=rz[:, 0:w],
                                                 in_=zadj[:, 0:w])
                ab = attp.tile([128, 512], BF16, name=f"ab{s}_{qc}_{p}",
                               tag=f"ab{p}")
                nc.vector.tensor_mul(ab[:, 0:w], attn[p][:, 0:w], rz[:, 0:w])
                att_sb.append(ab)

            def wo_units(s, sc_tiles, qc, w, att_sb, kqr, ydma_eng):
                for ot in range(8):
                    yt = sc_tiles[ot % 2]
                    yoff = ((ot >> 1) % 2) * 512
                    for j in range(2):
                        for h in range(2):
                            nc.tensor.matmul(
                                out=yt[h * 64:(h + 1) * 64, yoff:yoff + w],
                                lhsT=wo[j][:, ot * 128 + h * 64:ot * 128 + (h + 1) * 64],
                                rhs=att_sb[j][:, 0:w],
                                start=(j == 0), stop=(j == 1),
                                skip_group_check=True)
                    ysb = ystp.tile([128, 512], BF16,
                                    name=f"ysb{s}_{qc}_{ot}", tag="ysb")
                    nc.vector.tensor_mul(ysb[:, 0:w],
                                         yt[:, yoff:yoff + w],
                                         kqr[:, 0:w])
                    ydma_eng.dma_start(
                        out=d_out[s].ap()[ot * 128:(ot + 1) * 128,
                                          qc * 512:qc * 512 + w],
                        in_=ysb[:, 0:w])
                    yield "wo"

            def load_kqr(s, qc, w):
                kqr = small.tile([128, 512], F32, name=f"kqr{s}_{qc}",
                                 tag="kqr")
                nc.gpsimd.dma_start(
                    out=kqr[:, 0:w],
                    in_=bass.AP(tensor=d_in[f"kq{s}"], offset=qc * 512,
                                ap=[[0, 128], [1, w]]))
                return kqr

            def emit_attention_seq(s, sc_tiles, pat, pz, feeder=None):
                """Sequential-p, unfused exp, 2 score tiles; previous qc's
                Wo and external proj work interleaved via feed()."""
                sc3 = [t.tensor.reshape([128, 2, 512]) for t in sc_tiles]
                wo_pend = [None]

                def feed(n):
                    for _ in range(n):
                        if wo_pend[0] is not None:
                            if next(wo_pend[0], None) is not None:
                                continue
                            wo_pend[0] = None
                        if feeder is not None:
                            next(feeder, None)

                for qc in range(NQ[s]):
                    w = q_sc_width(s, qc)
                    kqr = load_kqr(s, qc, w)
                    att_sb = []
                    emit_scores = mk_scores(s, sc_tiles, qc, w, 512)
                    for p in range(2):
                        attn = {p: pat.tile([128, 512], F32,
                                            name=f"at{s}_{qc}_{p}", tag="at")}
                        zps = {p: pz.tile([128, 512], F32,
                                          name=f"z{s}_{qc}_{p}", tag="z")}
                        emit_pvz = mk_pvz(s, w, attn, zps, 512)

                        def emit_exp(kc):
                            pr = probsp.tile([128, 1024], BF16,
                                             name=f"pr{s}_{qc}_{p}_{kc}",
                                             tag="pr")
                            pr3 = pr.tensor.reshape([128, 2, 512])
                            sl = kc % 2
                            nc.scalar.activation(
                                out=pr3[:, :, 0:w],
                                in_=sc3[sl][:, :, 0:w],
                                func=mybir.ActivationFunctionType.Exp,
                                scale=SCALE)
                            return pr

                        emit_scores(p, 0, 0, 0)
                        pr_cur = emit_exp(0)
                        for kc in range(NK[s]):
                            pr = pr_cur
                            if kc + 1 < NK[s]:
                                emit_scores(p, kc + 1, (kc + 1) % 2, 0)
                                pr_cur = emit_exp(kc + 1)
                            emit_pvz(p, kc, pr, 0)
                            feed(2)
                        emit_norm(s, qc, p, w, attn, zps, att_sb)
                        feed(1)
                    while wo_pend[0] is not None:
                        feed(1)
                    wo_pend[0] = wo_units(s, sc_tiles, qc, w, att_sb, kqr,
                                          nc.gpsimd)
                while wo_pend[0] is not None:
                    feed(1)

            def emit_attention_ilv(s, sc_tiles, pat, pz):
                """Both head-pair chains interleaved so ScalarE stays
                packed; score tile p belongs to chain p; previous qc's Wo
                interleaved into the kc loop."""
                sc3 = [t.tensor.reshape([128, 2, 512]) for t in sc_tiles]
                wo_pend = [None]

                def feed(n):
                    for _ in range(n):
                        if wo_pend[0] is None:
                            return
                        if next(wo_pend[0], None) is None:
                            wo_pend[0] = None

                for qc in range(NQ[s]):
                    w = q_sc_width(s, qc)
                    kqr = load_kqr(s, qc, w)
                    att_sb = []
                    emit_scores = mk_scores(s, sc_tiles, qc, w, 512)
                    attn = {p: pat.tile([128, 512], F32,
                                        name=f"at{s}_{qc}_{p}", tag=f"at{p}")
                            for p in range(2)}
                    zps = {p: pz.tile([128, 512], F32,
                                      name=f"z{s}_{qc}_{p}", tag=f"z{p}")
                           for p in range(2)}
                    emit_pvz = mk_pvz(s, w, attn, zps, 512)

                    # chain p owns score tile p; two exp streams
                    def emit_exp1(p, kc):
                        pr = probsp.tile([128, 1024], BF16,
                                         name=f"pr{s}_{qc}_{p}_{kc}",
                                         tag="pr")
                        pr3 = pr.tensor.reshape([128, 2, 512])
                        nc.scalar.activation(
                            out=pr3[:, :, 0:w],
                            in_=sc3[p][:, :, 0:w],
                            func=mybir.ActivationFunctionType.Exp,
                            scale=SCALE)
                        return pr

                    pr_cur = {}
                    for p in range(2):
                        emit_scores(p, 0, p, 0)
                        pr_cur[p] = emit_exp1(p, 0)
                    for kc in range(NK[s]):
                        pr = {0: pr_cur[0], 1: pr_cur[1]}
                        for p in range(2):
                            if kc + 1 < NK[s]:
                                emit_scores(p, kc + 1, p, 0)
                                pr_cur[p] = emit_exp1(p, kc + 1)
                            emit_pvz(p, kc, pr[p], 0)
                        feed(1)
                    for p in range(2):
                        emit_norm(s, qc, p, w, attn, zps, att_sb)
                    while wo_pend[0] is not None:
                        feed(1)
                    wo_pend[0] = wo_units(s, sc_tiles, qc, w, att_sb, kqr,
                                          nc.sync)
                while wo_pend[0] is not None:
                    feed(1)

            # phase A: slot-0 k/v projections, ic-outer, full PSUM scope
            with tc.tile_pool(name="pproj", bufs=1, space="PSUM") as pproj:
                emit_proj0_eager(pproj)
            # phase B: slot-0 q proj eagerly driven so attention can start,
            # then slot-0 attention with the rest of slot-0 q proj + all
            # slot-1 proj fed in; 4 + 1 + 1 + 2 = 8 banks
            with tc.tile_pool(name="pscB", bufs=1, space="PSUM") as pscB, \
                 tc.tile_pool(name="patB", bufs=1, space="PSUM") as patB, \
                 tc.tile_pool(name="pzB", bufs=1, space="PSUM") as pzB, \
                 tc.tile_pool(name="ppj1", bufs=1, space="PSUM") as ppj1:
                scB = [pscB.tile([128, 1024], F32, name=f"scB{i}", tag=f"scB{i}")
                       for i in range(2)]
                f0 = proj_units(0, ppj1, kinds=("q",))
                for label in f0:
                    if label == "q:1:0":
                        break

                def chain2(a, b):
                    yield from a
                    yield from b

                feeder = chain2(f0, proj_units(1, ppj1))
                emit_attention_seq(0, scB, patB, pzB, feeder)
                for _ in feeder:  # drain remaining proj work
                    pass
            # phase C: slot-1 attention, p-chains interleaved (4 + 2 + 2 = 8)
            with tc.tile_pool(name="pscC", bufs=1, space="PSUM") as pscC, \
                 tc.tile_pool(name="patC", bufs=1, space="PSUM") as patC, \
                 tc.tile_pool(name="pzC", bufs=1, space="PSUM") as pzC:
                scC = [pscC.tile([128, 1024], F32, name=f"scC{i}", tag=f"scC{i}")
                       for i in range(2)]
                emit_attention_ilv(1, scC, patC, pzC)
    nc.compile()
    return nc


def _get_program(NQ, NK, W):
    key = (tuple(NQ), tuple(NK), tuple(W))
    if key not in _prog_cache:
        _prog_cache[key] = _build_program(list(NQ), list(NK), list(W))
    return _prog_cache[key]


def kernel(value, key, query, padding_mask, Wq, Wk, Wv, Wo):
    value = np.asarray(value)
    key = np.asarray(key)
    query = np.asarray(query)
    padding_mask = np.asarray(padding_mask)
    Wq, Wk, Wv, Wo = (np.asarray(a) for a in (Wq, Wk, Wv, Wo))

    lengths = (~padding_mask).sum(axis=0).astype(int)  # (B,)

    # --- batch pairing: assign batches to (group, slot) minimizing baked work ---
    def slot_counts(assign):
        lm = [max(int(lengths[assign[g][sl]]) for g in range(2))
              for sl in range(2)]
        nq = [(l + 511) // 512 for l in lm]
        nk = [(l + 127) // 128 for l in lm]
        w = [min(512, ((l - (q - 1) * 512 + 63) // 64) * 64)
             for l, q in zip(lm, nq)]
        return nq, nk, w

    best = None
    for perm in permutations(range(B)):
        a = ((perm[0], perm[1]), (perm[2], perm[3]))
        nq, nk, w = slot_counts(a)
        c = sum(k * 128 * ((q - 1) * 512 + ww) for q, k, ww in zip(nq, nk, w))
        if best is None or c < best[0]:
            best = (c, a)
    assign = best[1]
    nq, nk, w = slot_counts(assign)
    # slot 0 should be the smaller workload
    if nq[0] * nk[0] > nq[1] * nk[1]:
        assign = tuple((g[1], g[0]) for g in assign)
        nq, nk, w = slot_counts(assign)
    NQ, NK, W = nq, nk, w

    nc = _get_program(NQ, NK, W)

    # --- per-core inputs ---
    WqT = np.ascontiguousarray(Wq.T).astype(NPBF16)
    WkT = np.ascontiguousarray(Wk.T).astype(NPBF16)
    WvT = np.ascontiguousarray(Wv.T).astype(NPBF16)
    WoT = np.ascontiguousarray(Wo.T).astype(NPBF16)

    batch_qT, batch_kT, batch_vT, batch_kq = {}, {}, {}, {}
    for b in range(B):
        batch_qT[b] = np.ascontiguousarray(query[:, b, :].T).astype(NPBF16)
        batch_kT[b] = np.ascontiguousarray(key[:, b, :].T).astype(NPBF16)
        batch_vT[b] = np.ascontiguousarray(value[:, b, :].T).astype(NPBF16)
        batch_kq[b] = (np.arange(S).reshape(4, 512) < lengths[b]).astype(np.float32)

    in_maps = []
    for c in range(N_CORES):
        g, hq = c // 4, c % 4
        f0 = hq * 256
        m = {
            "wqT": np.ascontiguousarray(WqT[:, f0:f0 + 256]),
            "wkT": np.ascontiguousarray(WkT[:, f0:f0 + 256]),
            "wvT": np.ascontiguousarray(WvT[:, f0:f0 + 256]),
            "woT": np.ascontiguousarray(WoT[f0:f0 + 256, :]),
        }
        for sl in range(2):
            b = assign[g][sl]
            m[f"qT{sl}"] = batch_qT[b]
            m[f"kT{sl}"] = batch_kT[b]
            m[f"vT{sl}"] = batch_vT[b]
            m[f"kq{sl}"] = batch_kq[b]
            m[f"pd{sl}"] = np.full((1, 1), NK[sl] * 128 - int(lengths[b]),
                                   dtype=np.float32)
        in_maps.append(m)

    res = run_bass_kernel_spmd(nc, in_maps, list(range(N_CORES)))

    # --- gather: sum 4 head-quad partials per batch, transpose ---
    out = np.zeros((S, B, H), dtype=np.float32)
    for g in range(2):
        for sl in range(2):
            b = assign[g][sl]
            acc = np.zeros((H, S), dtype=np.float32)
            for hq in range(4):
                c = g * 4 + hq
                acc += res.results[c][f"y{sl}"].astype(np.float32)
            out[:, b, :] = acc.T
    return out
